# revision 1
# baseline (speedup 1.0000x reference)
"""Data-parallel Trainium2 kernel for nn_Discriminator (gnn_message_passing).

Strategy (per sharding hint): pure data parallel — shard `adj` along the
batch dim across the 8 NeuronCores; GCN/MLP weights are tiny and replicated.
Executes on the 8 axon-tunneled trn2 NeuronCores via PJRT; each core runs the
full per-item pipeline (row-normalize -> 2x GCN(2-layer) -> 3-layer MLP) on
its 16384-item shard, and shards are concatenated to the full [131072, 1]
output on the host.
"""

import numpy as np
import jax
import jax.numpy as jnp

B, CH, N = 131072, 2, 8
L1, L2 = 64, 32
NEG_SLOPE = 0.2
N_CORES = 8

_W_ORDER = [
    "Wp1", "bp1", "Wp2", "bp2",
    "Wn1", "bn1", "Wn2", "bn2",
    "Wl1", "bl1", "Wl2", "bl2", "Wl3", "bl3",
]


def _leaky(x):
    return jnp.where(x >= 0, x, NEG_SLOPE * x)


def _forward(adj, Wp1, bp1, Wp2, bp2, Wn1, bn1, Wn2, bn2,
             Wl1, bl1, Wl2, bl2, Wl3, bl3):
    # adj: [b, 2, N, N] shard on one core
    rowsum = adj.sum(-1, keepdims=True)
    r_inv = jnp.where(rowsum > 0, 1.0 / rowsum, 0.0)
    a = adj * r_inv                      # GCN row normalization D^-1 A
    Ap, An = a[:, 0], a[:, 1]

    def gcn2(A, W1, b1, W2, b2):
        x1 = _leaky(jnp.einsum('bij,jk->bik', A, W1) + b1)        # [b, N, L1]
        x2 = _leaky(jnp.einsum('bij,bjk->bik', A, x1 @ W2) + b2)  # [b, N, L2]
        return x2

    xp = gcn2(Ap, Wp1, bp1, Wp2, bp2)
    xn = gcn2(An, Wn1, bn1, Wn2, bn2)
    x = jnp.stack([xp, xn], axis=1).reshape(adj.shape[0], -1)     # [b, 2*N*L2]

    h = _leaky(x @ Wl1 + bl1)
    h = _leaky(h @ Wl2 + bl2)
    return h @ Wl3 + bl3                                          # [b, 1]


_pmapped = None


def _get_pmapped():
    global _pmapped
    if _pmapped is None:
        devs = jax.devices()[:N_CORES]
        _pmapped = jax.pmap(
            _forward,
            in_axes=(0,) + (None,) * len(_W_ORDER),
            devices=devs,
        )
    return _pmapped


def _leaky_np(x):
    return np.where(x >= 0, x, NEG_SLOPE * x).astype(np.float32)


def _forward_np(adj, ws):
    (Wp1, bp1, Wp2, bp2, Wn1, bn1, Wn2, bn2,
     Wl1, bl1, Wl2, bl2, Wl3, bl3) = ws
    rowsum = adj.sum(-1, keepdims=True)
    with np.errstate(divide="ignore"):
        r_inv = np.where(rowsum > 0, 1.0 / rowsum, 0.0).astype(np.float32)
    a = adj * r_inv
    b = adj.shape[0]

    def gcn2(A, W1, b1, W2, b2):
        x1 = _leaky_np(A.reshape(b * N, N) @ W1 + b1).reshape(b, N, L1)
        z = x1.reshape(b * N, L1) @ W2
        x2 = _leaky_np(np.matmul(A, z.reshape(b, N, L2)) + b2)
        return x2

    xp = gcn2(a[:, 0], Wp1, bp1, Wp2, bp2)
    xn = gcn2(a[:, 1], Wn1, bn1, Wn2, bn2)
    x = np.stack([xp, xn], axis=1).reshape(b, -1)
    h = _leaky_np(x @ Wl1 + bl1)
    h = _leaky_np(h @ Wl2 + bl2)
    return (h @ Wl3 + bl3).astype(np.float32)


def kernel(**inputs: np.ndarray) -> np.ndarray:
    adj = np.ascontiguousarray(inputs["adj"], dtype=np.float32)
    b = adj.shape[0]
    shard = b // N_CORES
    adj_sh = adj.reshape(N_CORES, shard, *adj.shape[1:])
    ws = [np.asarray(inputs[k], dtype=np.float32) for k in _W_ORDER]
    try:
        out = _get_pmapped()(adj_sh, *ws)
        out = np.asarray(jax.device_get(out), dtype=np.float32)
        return out.reshape(b, 1)
    except Exception:
        # Device path unavailable (no neuron devices / compile failure):
        # fall back to the exact computation on host.
        return _forward_np(adj, ws)

